# revision 6
# baseline (speedup 1.0000x reference)
"""Multi-head causal self-attention kernel for 8 Trainium2 NeuronCores.

Reference computation (self-attention, `key` input unused):
    q = split_heads(query @ Wq.T); k = split_heads(query @ Wk.T)
    v = split_heads(query @ Wv.T)
    scores = (q @ k.T) / sqrt(hd);  causal mask;  w = softmax(scores)
    attn_out = merge_heads(w @ v) @ Wo.T
    returns (attn_out, w)

Sharding: 8 cores = 2 batches x 4 head-groups (4 heads each).
Per core everything is computed feature-on-partition (transposed):
  xT [DIN,S], QT/KT [256,S], V [S,256] (natural), scores twice:
  pass1 [q,k] tiles (masked exp + row-sum accumulate -> normalized weights out),
  pass2 [k,q] tiles (masked exp -> moving operand of attn@V),
  O.T [64,S] per head normalized via a broadcast tile, out-projection row-sharded.
Host reassembles: transpose partial outputs, sum over head-groups, concat weights.
Only the causal lower triangle is ever written to the weights output; the
upper triangle stays zero because outputs are donated zero-initialized buffers.
"""

import sys

for _p in ("/opt/trn_rl_repo", "/root/.axon_site/_ro/trn_rl_repo"):
    if _p not in sys.path:
        sys.path.insert(0, _p)

import numpy as np

import concourse.bass as bass
import concourse.tile as tile
from concourse import bacc, mybir
from concourse.bass_utils import run_bass_kernel_spmd

F32 = mybir.dt.float32
F32R = mybir.dt.float32r

N_CORES = 8
NEG = -1e30

# Set by test.py to collect a hardware profile.
PROFILE = False
LAST_RESULTS = None


def build_program(S=2048, DIN=1024, NHC=4, DH=64, piece=1024, use_f32r=True,
                  n_cores=N_CORES):
    """Build + compile the per-core Bass program (identical on all cores)."""
    MMDT = F32R if use_f32r else F32
    DCAT = NHC * DH            # per-core concat head dim (256)
    NQ = S // 128              # number of 128-row seq tiles
    NT = DIN // 128            # number of 128-row feature tiles
    assert DH == 64 and NHC % 2 == 0 and S % 512 == 0 and DIN % 128 == 0

    nc = bacc.Bacc("TRN2", target_bir_lowering=False, debug=False,
                   num_devices=n_cores)

    xT_d = nc.dram_tensor("xT", [DIN, S], MMDT, kind="ExternalInput").ap()
    wqT_d = nc.dram_tensor("wqT", [DIN, DCAT], MMDT, kind="ExternalInput").ap()
    wkT_d = nc.dram_tensor("wkT", [DIN, DCAT], MMDT, kind="ExternalInput").ap()
    wvT_d = nc.dram_tensor("wvT", [DIN, DCAT], MMDT, kind="ExternalInput").ap()
    woT_d = nc.dram_tensor("woT", [DCAT, DIN], MMDT, kind="ExternalInput").ap()
    maskU_d = nc.dram_tensor("maskU", [128, 128], F32, kind="ExternalInput").ap()
    maskL_d = nc.dram_tensor("maskL", [128, 128], F32, kind="ExternalInput").ap()
    ident_d = nc.dram_tensor("ident", [128, 128], F32, kind="ExternalInput").ap()

    attnw_d = nc.dram_tensor("attnw", [NHC, S, S], F32, kind="ExternalOutput").ap()
    outT_d = nc.dram_tensor("outT", [DIN, S], F32, kind="ExternalOutput").ap()
    # scratch for spreading per-head 1/denom rows across partitions
    rscr = nc.dram_tensor("rscr", [NHC, S], F32)
    rscr_d = rscr.ap()

    scale = 1.0 / np.sqrt(DH)

    with tile.TileContext(nc) as tc:
        with tc.tile_pool(name="consts", bufs=1) as consts, \
             tc.tile_pool(name="persist", bufs=1) as persist:
            maskU = consts.tile([128, 128], F32, tag="maskU")
            maskL = consts.tile([128, 128], F32, tag="maskL")
            ident = consts.tile([128, 128], F32, tag="ident")
            nc.sync.dma_start(out=maskU, in_=maskU_d)
            nc.sync.dma_start(out=maskL, in_=maskL_d)
            nc.sync.dma_start(out=ident, in_=ident_d)

            # persistent activations
            QT = persist.tile([128, DCAT // 128, S], MMDT, tag="QT")
            KT = persist.tile([128, DCAT // 128, S], MMDT, tag="KT")
            V = persist.tile([128, NQ, DCAT], MMDT, tag="V")
            OT = persist.tile([128, DCAT // 128, S], MMDT, tag="OT")
            woTs = persist.tile([128, DCAT // 128, DIN], MMDT, tag="woTs")
            for t in range(DCAT // 128):
                nc.sync.dma_start(out=woTs[:, t, :], in_=woT_d[128 * t:128 * t + 128, :])

            # ---------------- phase 1: projections ----------------
            with tc.tile_pool(name="ph1", bufs=1) as ph1, \
                 tc.tile_pool(name="psA", bufs=3, space="PSUM") as psA:
                xTs = ph1.tile([128, NT, S], MMDT, tag="xTs")
                wq = ph1.tile([128, NT, DCAT], MMDT, tag="wq")
                wk = ph1.tile([128, NT, DCAT], MMDT, tag="wk")
                wv = ph1.tile([128, NT, DCAT], MMDT, tag="wv")
                for t in range(NT):
                    nc.sync.dma_start(out=xTs[:, t, :], in_=xT_d[128 * t:128 * t + 128, :])
                    nc.sync.dma_start(out=wq[:, t, :], in_=wqT_d[128 * t:128 * t + 128, :])
                    nc.sync.dma_start(out=wk[:, t, :], in_=wkT_d[128 * t:128 * t + 128, :])
                    nc.sync.dma_start(out=wv[:, t, :], in_=wvT_d[128 * t:128 * t + 128, :])

                # QT / KT: [dq, s] = sum_din w?T[din, dq].T @ xT[din, s]
                for dst, w in ((QT, wq), (KT, wk)):
                    for m in range(DCAT // 128):
                        for n0 in range(0, S, 512):
                            ps = psA.tile([128, 512], F32, tag="psA")
                            for t in range(NT):
                                nc.tensor.matmul(
                                    ps,
                                    (w[:, t, 128 * m:128 * m + 128]),
                                    (xTs[:, t, n0:n0 + 512]),
                                    start=(t == 0), stop=(t == NT - 1))
                            nc.scalar.copy(out=dst[:, m, n0:n0 + 512], in_=ps)
                # V: [s, dv] = sum_din xT[din, s].T @ wvT[din, dv]
                for si in range(NQ):
                    ps = psA.tile([128, DCAT], F32, tag="psV")
                    for t in range(NT):
                        nc.tensor.matmul(
                            ps,
                            (xTs[:, t, 128 * si:128 * si + 128]),
                            (wv[:, t, :]),
                            start=(t == 0), stop=(t == NT - 1))
                    nc.scalar.copy(out=V[:, si, :], in_=ps)

            # ---------------- phase 2: per-head attention ----------------
            for h in range(NHC):
                hp = 64 * (h % 2)          # partition offset of this head
                hm = h // 2                # which 128-wide tile
                QTh = QT[hp:hp + 64, hm, :]
                KTh = KT[hp:hp + 64, hm, :]

                with tc.tile_pool(name=f"hd{h}", bufs=1) as hd, \
                     tc.tile_pool(name=f"wrow{h}", bufs=3) as wrow, \
                     tc.tile_pool(name=f"ps2_{h}", bufs=2, space="PSUM") as ps2, \
                     tc.tile_pool(name=f"psS_{h}", bufs=2, space="PSUM") as psS, \
                     tc.tile_pool(name=f"psT_{h}", bufs=1, space="PSUM") as psT, \
                     tc.tile_pool(name=f"dsml{h}", bufs=4) as dsml:

                    recip_h = hd.tile([128, NQ], F32, tag="recip")

                    # ---- pass 1: scores[q,k], exp+rowsum, normalize, DMA out
                    for qi in range(NQ):
                        kext = 128 * (qi + 1)
                        wr = wrow.tile([128, S], F32, tag="wr")
                        den = dsml.tile([128, 2], F32, tag="den")
                        npieces = (kext + piece - 1) // piece
                        for ip in range(npieces):
                            c0 = ip * piece
                            w = min(piece, kext - c0)
                            ps = ps2.tile([128, piece], F32, tag="row")
                            for n0 in range(0, w, 512):
                                nn = min(512, w - n0)
                                nc.tensor.matmul(
                                    ps[:, n0:n0 + nn],
                                    (QTh[:, 128 * qi:128 * qi + 128]),
                                    (KTh[:, c0 + n0:c0 + n0 + nn]),
                                    start=True, stop=True)
                            # causal mask on the diagonal 128x128 block
                            d0 = 128 * qi
                            if c0 <= d0 < c0 + w:
                                nc.vector.tensor_add(
                                    ps[:, d0 - c0:d0 - c0 + 128],
                                    ps[:, d0 - c0:d0 - c0 + 128], maskU)
                            nc.scalar.activation(
                                out=wr[:, c0:c0 + w], in_=ps[:, :w],
                                func=mybir.ActivationFunctionType.Exp,
                                scale=scale, accum_out=den[:, ip:ip + 1])
                        if npieces == 2:
                            nc.vector.tensor_add(den[:, 0:1], den[:, 0:1], den[:, 1:2])
                        nc.vector.reciprocal(recip_h[:, qi:qi + 1], den[:, 0:1])
                        nc.vector.tensor_scalar_mul(
                            wr[:, :kext], wr[:, :kext], recip_h[:, qi:qi + 1])
                        nc.sync.dma_start(
                            out=attnw_d[h, 128 * qi:128 * qi + 128, 0:kext],
                            in_=wr[:, :kext])

                    # ---- pass 2: scores.T[k,q] -> exp -> WT rows (kept in SBUF)
                    wt_rows = []
                    for ki in range(NQ):
                        q0 = 128 * ki
                        width = S - q0
                        wt = hd.tile([128, width], MMDT, tag=f"wt{ki}")
                        wt_rows.append(wt)
                        npieces = (width + piece - 1) // piece
                        for ip in range(npieces):
                            c0 = ip * piece
                            w = min(piece, width - c0)
                            ps = ps2.tile([128, piece], F32, tag="row")
                            for n0 in range(0, w, 512):
                                nn = min(512, w - n0)
                                nc.tensor.matmul(
                                    ps[:, n0:n0 + nn],
                                    (KTh[:, 128 * ki:128 * ki + 128]),
                                    (QTh[:, q0 + c0 + n0:q0 + c0 + n0 + nn]),
                                    start=True, stop=True)
                            if ip == 0:
                                # diagonal block is the first 128 cols of this row
                                nc.vector.tensor_add(ps[:, 0:128], ps[:, 0:128], maskL)
                            nc.scalar.activation(
                                out=wt[:, c0:c0 + w], in_=ps[:, :w],
                                func=mybir.ActivationFunctionType.Exp, scale=scale)

                    # ---- 1/denom broadcast tile: [p, q] = recip[q]
                    pst = psT.tile([16 if NQ <= 16 else NQ, 128], F32, tag="pst")
                    nc.tensor.transpose(pst[:NQ, :], recip_h[:, :NQ], ident)
                    rrow = dsml.tile([NQ, 128], F32, tag="rrow")
                    nc.vector.tensor_copy(rrow, pst[:NQ, :])
                    nc.sync.dma_start(
                        out=rscr_d[h].rearrange("(a b) -> a b", a=NQ), in_=rrow)
                    bcast = hd.tile([128, S], F32, tag="bcast")
                    nc.sync.dma_start(
                        out=bcast,
                        in_=bass.AP(tensor=rscr, offset=h * S, ap=[[0, 128], [1, S]]))

                    # ---- attn @ V: O.T[d, q] accumulated over k tiles
                    for jc in range(S // 512):
                        pso = psS.tile([64, 512], F32, tag="pso")
                        last = min(4 * jc + 3, NQ - 1)
                        for ki in range(last + 1):
                            if 128 * ki <= 512 * jc:
                                nc.tensor.matmul(
                                    pso,
                                    (V[:, ki, 64 * h:64 * h + 64]),
                                    (wt_rows[ki][:, 512 * jc - 128 * ki:
                                                          512 * jc - 128 * ki + 512]),
                                    start=(ki == 0), stop=(ki == last))
                            else:
                                off = 128 * ki - 512 * jc
                                nc.tensor.matmul(
                                    pso[:, off:],
                                    (V[:, ki, 64 * h:64 * h + 64]),
                                    (wt_rows[ki][:, 0:512 - off]),
                                    start=False, stop=(ki == last))
                        nc.vector.tensor_mul(
                            OT[hp:hp + 64, hm, 512 * jc:512 * jc + 512],
                            pso, bcast[0:64, 512 * jc:512 * jc + 512])

            # ---------------- phase 3: out-projection ----------------
            with tc.tile_pool(name="ph3", bufs=4) as ph3, \
                 tc.tile_pool(name="psC", bufs=4, space="PSUM") as psC:
                for m in range(DIN // 128):
                    for n0 in range(0, S, 512):
                        ps = psC.tile([128, 512], F32, tag="psC")
                        for t in range(DCAT // 128):
                            nc.tensor.matmul(
                                ps,
                                (woTs[:, t, 128 * m:128 * m + 128]),
                                (OT[:, t, n0:n0 + 512]),
                                start=(t == 0), stop=(t == DCAT // 128 - 1))
                        ot = ph3.tile([128, 512], F32, tag="ot")
                        nc.scalar.copy(out=ot, in_=ps)
                        nc.sync.dma_start(
                            out=outT_d[128 * m:128 * m + 128, n0:n0 + 512], in_=ot)

    nc.compile()
    return nc


_PROGRAM_CACHE = {}


def _get_program(**kw):
    key = tuple(sorted(kw.items()))
    if key not in _PROGRAM_CACHE:
        _PROGRAM_CACHE[key] = build_program(**kw)
    return _PROGRAM_CACHE[key]


def make_core_inputs(query, Wq, Wk, Wv, Wo, S, DIN, NHC, DH, n_cores=N_CORES):
    """Shard full inputs into per-core input maps."""
    DCAT = NHC * DH
    ng = n_cores * DCAT // Wq.shape[0] and (Wq.shape[0] // DCAT)  # head groups
    ng = Wq.shape[0] // DCAT
    maskU = np.where(np.arange(128)[None, :] > np.arange(128)[:, None],
                     np.float32(NEG), np.float32(0.0))
    maskL = np.ascontiguousarray(maskU.T)
    ident = np.eye(128, dtype=np.float32)
    in_maps = []
    for c in range(n_cores):
        b, hg = divmod(c, ng)
        sl = slice(DCAT * hg, DCAT * hg + DCAT)
        in_maps.append({
            "xT": np.ascontiguousarray(query[b].T),
            "wqT": np.ascontiguousarray(Wq[sl, :].T),
            "wkT": np.ascontiguousarray(Wk[sl, :].T),
            "wvT": np.ascontiguousarray(Wv[sl, :].T),
            "woT": np.ascontiguousarray(Wo[:, sl].T),
            "maskU": maskU, "maskL": maskL, "ident": ident,
        })
    return in_maps


def assemble(results, B, NH, S, DIN, NHC, n_cores=N_CORES):
    ng = NH // NHC
    attn_out = np.zeros((B, S, DIN), dtype=np.float32)
    attn_w = np.empty((B, NH, S, S), dtype=np.float32)
    for c in range(n_cores):
        b, hg = divmod(c, ng)
        attn_out[b] += results[c]["outT"].T
        attn_w[b, NHC * hg:NHC * hg + NHC] = results[c]["attnw"]
    return attn_out, attn_w


def kernel(query, key, attention_mask, Wq, Wk, Wv, Wo):
    global LAST_RESULTS
    query = np.asarray(query, dtype=np.float32)
    Wq = np.asarray(Wq, dtype=np.float32)
    Wk = np.asarray(Wk, dtype=np.float32)
    Wv = np.asarray(Wv, dtype=np.float32)
    Wo = np.asarray(Wo, dtype=np.float32)

    B, S, DIN = query.shape
    NH = 16 if DIN == 1024 else DIN // 64
    DH = DIN // NH
    ng = N_CORES // B                 # head groups per batch
    NHC = NH // ng                    # heads per core

    nc = _get_program(S=S, DIN=DIN, NHC=NHC, DH=DH)
    in_maps = make_core_inputs(query, Wq, Wk, Wv, Wo, S, DIN, NHC, DH)
    res = run_bass_kernel_spmd(nc, in_maps, core_ids=list(range(N_CORES)),
                               trace=PROFILE)
    LAST_RESULTS = res
    return assemble(res.results, B, NH, S, DIN, NHC)
